# revision 28
# baseline (speedup 1.0000x reference)
"""Trainium2 Bass kernel for batched shared-query attention.

Problem:
  query [S=128, D=64] shared across all (b, w);
  keys/values [B=64, W=32, T=256, D=64];
  out[b, w] = softmax(query @ keys[b, w].T, axis=-1) @ values[b, w].

Strategy (8 NeuronCores, data-parallel over B). Memory-bound target:
per-core HBM traffic is ~98K DMA descriptors (K 32K + V 32K + out 32K)
at a ~20ns/descriptor floor across 16 DMA engines ~= 125us. So:
  - K and V are loaded t-pair-interleaved: partition p holds rows
    t=2p and t=2p+1 (512B contiguous descriptors).
  - V rides the Activation HWDGE queue (K + out ride qSP) so the two
    hardware DGE queues cover each other's inter-instruction gaps.
  - All heavy matmuls run at >= f32r rate: QK^T path in f32r (fp32
    bits, fast PE mode, N=256 per pair), E@V path in bf16 (E comes out
    of the exp activation as bf16 for free; V is cast fp32->bf16 split
    across the otherwise-idle GpSimd (even-t half) and Scalar (odd-t
    half) engines, keeping DVE for the kt copy + normalization).
  - exp(p)/sum(exp(p)) without max-subtraction is safe: |p| <= ~50 so
    exp stays in fp32 range (and the reference's p==0 mask never fires
    for randn inputs).  The softmax denominator rides as a 65th "ones"
    column in the bf16 V tile, so each out-matmul also emits the
    denominator; a DVE reciprocal+broadcast-mul normalizes.
  - DMA batching: 8-pair super-groups (1 dma_start each for K, V, out);
    compute runs in 4-pair groups (PSUM: 2 transpose banks + 4 pT
    banks + 2 out banks; start/stop accumulation flags are tracked per
    2KB PSUM bank).
  - Software pipelining (2-group skew): iteration i runs transposes(i)
    on PE, pT-matmuls(i-1), out-matmuls(i-2); exp(i-1) on ACT and the
    kt PSUM->SBUF copy(i) on DVE overlap PE work of other groups, so
    the ~1.1us exp never sits on the PE critical path.
"""

import sys

sys.path.insert(0, "/opt/trn_rl_repo")

import numpy as np

import concourse.bass as bass
from concourse import bacc
import concourse.mybir as mybir
import concourse.tile as tile
from concourse.bass_utils import run_bass_kernel_spmd
from concourse.masks import make_identity

F32 = mybir.dt.float32
F32R = mybir.dt.float32r
BF16 = mybir.dt.bfloat16
N_CORES = 8
B, W, T, S, D = 64, 32, 256, 128, 64
B_PER = B // N_CORES
G = 4          # (b, w) pairs per compute group
SUP = 8        # pairs per DMA super-group
LOOKAHEAD = 3  # super-groups of K/V prefetch


def build_bass(b_per=B_PER, w=W, pt_f32r=True, il_out=True):
    nc = bacc.Bacc()
    q_t = nc.declare_dram_parameter("query", [S, D], F32, isOutput=False)
    k_t = nc.declare_dram_parameter("keys", [b_per, w, T, D], F32, isOutput=False)
    v_t = nc.declare_dram_parameter("values", [b_per, w, T, D], F32, isOutput=False)
    o_t = nc.declare_dram_parameter("out", [b_per, w, S, D], F32, isOutput=True)

    EXP = mybir.ActivationFunctionType.Exp
    # f32r is a real rounding format: every producer feeding an f32r
    # matmul must declare f32r output, so the whole QK^T path (identity,
    # K tiles, transpose PSUM, kt, qz) is typed f32r in that mode.
    QZ_DT = F32R if pt_f32r else BF16
    KV_DT = F32R if pt_f32r else F32

    n_groups = b_per * w // G          # 64
    n_supers = n_groups * G // SUP
    sup_per_b = w // SUP
    GPS = SUP // G                     # compute groups per super-group

    with tile.TileContext(nc) as tc:
        with tc.tile_pool(name="const", bufs=1) as const:
            if pt_f32r:
                # gpsimd/DVE can't write f32r directly; build the f32r
                # identity and qz via SBUF->SBUF DMA from f32 staging
                # (DMA is an accepted f32r producer).
                ident_q = const.tile([128, 128], F32)
                make_identity(nc, ident_q[:])
                ident = const.tile([128, 128], F32R)
                nc.sync.dma_start(
                    out=ident[:], in_=ident_q[:].bitcast(F32R)
                )
            else:
                ident = const.tile([128, 128], KV_DT)
                make_identity(nc, ident[:])
                ident_q = ident
            q_sb = const.tile([S, D], F32)
            nc.sync.dma_start(out=q_sb[:], in_=q_t[:, :])
            # qz [128, 256]: rows 0:64 cols 0:128 hold Qt; rows 64:128
            # cols 128:256 hold Qt again (contracts the odd-t half of the
            # stacked K^T); zeros elsewhere.  One N=256 matmul per pair
            # then yields [pT_even | pT_odd].
            qz = const.tile([128, 2 * S], QZ_DT)
            qzs = (
                const.tile([64, 2 * S], F32, name="qzs") if pt_f32r else None
            )
            if not pt_f32r:
                nc.vector.memset(qz[:], 0.0)

            with (
                tc.tile_pool(name="kc", bufs=LOOKAHEAD + 1) as kc_pool,
                tc.tile_pool(name="vc", bufs=LOOKAHEAD + 1) as vc_pool,
                tc.tile_pool(name="vb", bufs=4) as vb_pool,
                tc.tile_pool(name="kts", bufs=3) as kt_pool,
                tc.tile_pool(name="et", bufs=2) as et_pool,
                tc.tile_pool(name="osb", bufs=2) as os_pool,
                tc.tile_pool(name="rc", bufs=2) as rc_pool,
                tc.tile_pool(name="ktp", bufs=2, space="PSUM") as ktp_pool,
                tc.tile_pool(name="ptp", bufs=2, space="PSUM") as pt_pool,
                tc.tile_pool(name="cbp", bufs=2, space="PSUM") as cb_pool,
            ):
                sup_tiles = {}
                grp = {}
                out_tiles = {}

                def issue_load(s):
                    bb = s // sup_per_b
                    w0 = (s % sup_per_b) * SUP
                    kk = kc_pool.tile([128, SUP * 128], KV_DT)
                    k_src = k_t[bb, w0 : w0 + SUP].rearrange(
                        "g (p j) d -> p g j d", j=2
                    )
                    if pt_f32r:
                        k_src = k_src.bitcast(F32R)
                    nc.sync.dma_start(
                        out=kk[:].rearrange("p (g j d) -> p g j d", g=SUP, j=2),
                        in_=k_src,
                    )
                    # V rides the Activation HWDGE queue so the two HW DGE
                    # queues (qSP, qAct) keep the DMA engines fed through
                    # each other's inter-instruction gaps
                    vv = vc_pool.tile([128, SUP * 128], F32)
                    nc.scalar.dma_start(
                        out=vv[:].rearrange("p (g j d) -> p g j d", g=SUP, j=2),
                        in_=v_t[bb, w0 : w0 + SUP].rearrange(
                            "g (p j) d -> p g j d", j=2
                        ),
                    )
                    sup_tiles[s] = (kk, vv)

                for s in range(min(LOOKAHEAD, n_supers)):
                    issue_load(s)

                # ---- Q setup (after the prologue loads so the big K/V
                # DMAs hit the queues first; qz roundtrips ride the
                # scalar queue and overlap the first K/V transfers) ----
                # borrow a ptp-shaped buffer (same tag/size as the
                # loop's pt_ps tiles, so PSUM budget is unchanged)
                qt_full = pt_pool.tile([128, G * 256], F32, name="pt_ps")
                qt_ps = qt_full[0:64, 0:S]
                nc.tensor.matmul(
                    qt_ps, q_sb[:], ident_q[:],
                    is_transpose=True, start=True, stop=True,
                )
                # With il_out, Qt's s-columns are reordered to
                # s' = k*64 + s//2 (k = s parity) so the out-matmul
                # tiles come out s-pair-packed -> 512B output DMA runs.
                qt_src = qt_ps
                qz_dst = qzs[:, 0:S] if pt_f32r else qz[0:64, 0:S]
                if il_out:
                    qt_src = qt_ps.rearrange("p (s2 k) -> p k s2", k=2)
                    qz_dst = qz_dst.rearrange("p (k s2) -> p k s2", k=2)
                if pt_f32r:
                    # f32 staging: [Qt | zeros], then SBUF->SBUF DMAs
                    # place [Qt|0] on rows 0:64 and [0|Qt] on rows
                    # 64:128 of the f32r qz (DMA is an accepted f32r
                    # producer; DVE/ACT are not).
                    nc.vector.memset(qzs[:, S : 2 * S], 0.0)
                    nc.scalar.copy(qz_dst, qt_src)
                    nc.scalar.dma_start(
                        out=qz[0:64, :], in_=qzs[:, :].bitcast(F32R)
                    )
                    nc.scalar.dma_start(
                        out=qz[64:128, S : 2 * S],
                        in_=qzs[:, 0:S].bitcast(F32R),
                    )
                    nc.scalar.dma_start(
                        out=qz[64:128, 0:S],
                        in_=qzs[:, S : 2 * S].bitcast(F32R),
                    )
                else:
                    nc.scalar.copy(qz_dst, qt_src)
                    nc.scalar.dma_start(
                        out=qz[64:128, S : 2 * S], in_=qz[0:64, 0:S]
                    )

                for i in range(n_groups + 2):
                    # ---- stage 1: loads, V cast, K transposes ----
                    if i < n_groups:
                        if i % GPS == 0 and i // GPS + LOOKAHEAD < n_supers:
                            issue_load(i // GPS + LOOKAHEAD)
                        s = i // GPS
                        half = i % GPS
                        k_sup, v_sup = sup_tiles[s]
                        # bf16 V with a ones column per (pair, parity):
                        # cols g*130 + j*65 + [0:64] = V_j, col 64 = 1.0
                        vb_t = vb_pool.tile([128, G * 130], BF16)
                        vb_v = vb_t[:].rearrange("p (g j c) -> p g j c", g=G, c=65)
                        nc.vector.memset(vb_v[:, :, :, 64:65], 1.0)
                        v_src = v_sup[
                            :, half * 512 : (half + 1) * 512
                        ].rearrange("p (g j d) -> p g j d", g=G, j=2)
                        # fp32->bf16 V cast split: even-t half on GpSimd,
                        # odd-t half on Scalar (both off DVE's critical
                        # path; ACT has slack next to the per-group exp)
                        nc.gpsimd.tensor_copy(
                            vb_v[:, :, 0, 0:64], v_src[:, :, 0]
                        )
                        nc.scalar.copy(
                            vb_v[:, :, 1, 0:64], v_src[:, :, 1]
                        )
                        # stacked K^T per pair: partitions 0:64 = K^T of
                        # even t's, 64:128 = odd t's (one 128x128 PE
                        # transpose per pair)
                        kt_ps = ktp_pool.tile([128, G * 128], KV_DT)
                        for g in range(G):
                            nc.tensor.matmul(
                                kt_ps[:, g * 128 : (g + 1) * 128],
                                k_sup[
                                    :,
                                    half * 512 + g * 128 : half * 512 + (g + 1) * 128,
                                ],
                                ident[:],
                                is_transpose=True,
                                start=(g == 0),
                                stop=(g == G - 1),
                            )
                        kt_sb = kt_pool.tile([128, G * 128], QZ_DT)
                        nc.vector.tensor_copy(kt_sb[:], kt_ps[:])
                        grp[i] = {"vb": vb_t, "kt": kt_sb}

                    # ---- stage 2: pT matmuls + exp ----
                    jg = i - 1
                    if 0 <= jg < n_groups:
                        gd = grp[jg]
                        pt_ps = pt_pool.tile([128, G * 256], F32)
                        # bank-alternating order; start/stop are tracked at
                        # PSUM-bank granularity (pairs 0,1 -> bank A, 2,3 ->
                        # bank B), so each bank's first write starts it
                        for g in (0, 2, 1, 3):
                            nc.tensor.matmul(
                                pt_ps[:, g * 256 : (g + 1) * 256],
                                gd["kt"][:, g * 128 : (g + 1) * 128],
                                qz[:],
                                start=(g % 2 == 0),
                                stop=(g % 2 == 1),
                            )
                        et_sb = et_pool.tile([128, G * 256], BF16)
                        nc.scalar.activation(et_sb[:], pt_ps[:], EXP)
                        gd["et"] = et_sb

                    # ---- stage 3: out matmuls + normalize (+ store) ----
                    m = i - 2
                    if 0 <= m < n_groups:
                        gd = grp[m]
                        et_sb = gd["et"]
                        vb_t = gd["vb"]
                        # out[s, v|den] += Et_j.T @ [V_j | 1]
                        if il_out:
                            # s-pair-packed: pair g -> partition half
                            # h=g%2 (PE column-tile (0,64)), col block
                            # c=g//2; within a block, s-parity k selects
                            # a 65-col region.  HW has_written bits are
                            # per element; the single start clears the
                            # bank.  (CoreSim's zero-region tracker
                            # mis-addresses partition-base-64 APs, hence
                            # skip_group_check; validated on HW.)
                            cb_t = cb_pool.tile([128, 2 * 130], F32)
                            n_mm = 0
                            for j in range(2):
                                for g in range(G):
                                    h, c = g % 2, g // 2
                                    for k in range(2):
                                        nc.tensor.matmul(
                                            cb_t[
                                                h * 64 : (h + 1) * 64,
                                                c * 130 + k * 65 : c * 130
                                                + k * 65
                                                + 65,
                                            ],
                                            et_sb[
                                                :,
                                                g * 256 + j * 128 + k * 64 : g * 256
                                                + j * 128
                                                + k * 64
                                                + 64,
                                            ],
                                            vb_t[
                                                :,
                                                g * 130 + j * 65 : g * 130
                                                + j * 65
                                                + 65,
                                            ],
                                            start=(
                                                j == 0 and g < 2 and k == 0
                                            ),
                                            stop=(
                                                j == 1 and g >= 2 and k == 1
                                            ),
                                            skip_group_check=True,
                                        )
                                        n_mm += 1
                            cb_v = cb_t[:].rearrange("p (c x) -> p c x", x=65)
                            nG = 2 * GPS
                        else:
                            cb_t = cb_pool.tile([128, G * 65], F32)
                            for j in range(2):
                                for g in range(G):
                                    nc.tensor.matmul(
                                        cb_t[:, g * 65 : (g + 1) * 65],
                                        et_sb[
                                            :,
                                            g * 256 + j * 128 : g * 256
                                            + j * 128
                                            + 128,
                                        ],
                                        vb_t[
                                            :,
                                            g * 130 + j * 65 : g * 130
                                            + j * 65
                                            + 65,
                                        ],
                                        start=(j == 0 and g == 0),
                                        stop=(j == 1 and g == G - 1),
                                    )
                            cb_v = cb_t[:].rearrange("p (g x) -> p g x", x=65)
                        # normalize: recip of the den columns, bcast mul
                        nden = 4
                        rc_t = rc_pool.tile([128, nden], F32)
                        nc.vector.reciprocal(
                            rc_t[:].rearrange("p (g o) -> p g o", o=1),
                            cb_v[:, :, 64:65],
                        )
                        if m % GPS == 0:
                            out_tiles[m // GPS] = os_pool.tile(
                                [128, SUP * 64], F32, name="os_t"
                            )
                        os_t = out_tiles[m // GPS]
                        os_v = os_t[
                            :, (m % GPS) * 256 : (m % GPS + 1) * 256
                        ].rearrange("p (g v) -> p g v", v=64)
                        nc.vector.tensor_mul(
                            os_v,
                            cb_v[:, :, 0:64],
                            rc_t[:]
                            .rearrange("p (g o) -> p g o", o=1)
                            .broadcast_to([128, 4, 64]),
                        )
                        if m % GPS == GPS - 1:
                            s_out = m // GPS
                            bb = s_out // sup_per_b
                            w0 = (s_out % sup_per_b) * SUP
                            if il_out:
                                # partition (h s2) holds s=2*s2+k rows of
                                # pair g = a*4 + c*2 + h; (k v) runs are
                                # 512B-contiguous in DRAM
                                nc.sync.dma_start(
                                    out=o_t[bb, w0 : w0 + SUP].rearrange(
                                        "(a c h) (s2 k) v -> h s2 a c k v",
                                        a=2, c=2, h=2, k=2,
                                    ),
                                    in_=os_t[:].rearrange(
                                        "(h s2) (a c k v) -> h s2 a c k v",
                                        h=2, a=2, c=2, k=2,
                                    ),
                                )
                            else:
                                nc.sync.dma_start(
                                    out=o_t[bb, w0 : w0 + SUP].rearrange(
                                        "g s v -> s g v"
                                    ),
                                    in_=os_t[:].rearrange(
                                        "p (g v) -> p g v", g=SUP
                                    ),
                                )
                        if m >= 2:
                            grp.pop(m - 2, None)
    nc.finalize()
    return nc


_NC_CACHE = {}
PT_F32R = True
IL_OUT = False


def _get_nc(b_per=B_PER, w=W):
    key = (b_per, w, PT_F32R, IL_OUT)
    if key not in _NC_CACHE:
        _NC_CACHE[key] = build_bass(
            b_per, w, pt_f32r=PT_F32R, il_out=IL_OUT
        )
    return _NC_CACHE[key]


def run(query, keys, values, trace=False):
    query = np.ascontiguousarray(np.asarray(query), dtype=np.float32)
    keys = np.ascontiguousarray(np.asarray(keys), dtype=np.float32)
    values = np.ascontiguousarray(np.asarray(values), dtype=np.float32)
    nc = _get_nc()
    in_maps = [
        {
            "query": query,
            "keys": keys[c * B_PER : (c + 1) * B_PER],
            "values": values[c * B_PER : (c + 1) * B_PER],
        }
        for c in range(N_CORES)
    ]
    res = run_bass_kernel_spmd(nc, in_maps, list(range(N_CORES)), trace=trace)
    out = np.concatenate(
        [np.asarray(res.results[c]["out"]) for c in range(N_CORES)], axis=0
    ).astype(np.float32)
    return out, res


def kernel(query, keys, values):
    out, _ = run(query, keys, values)
    return out
